# revision 13
# baseline (speedup 1.0000x reference)
"""Distributed cosine-similarity top-k retrieval kernel for 8 Trainium2 NeuronCores.

Strategy (sharding_hint: row-wise table sharding):
  - 999424 of the 1M table rows are L2-normalized and sharded row-wise across
    8 cores (124928 rows = 122 groups of 1024 candidates each); the 576
    remainder rows are scored exactly on the host and merged.
  - Each core streams its shard through the TensorEngine (bf16 matmul vs all
    256 queries -> fp32 scores in PSUM). PSUM is one [128, 4096] fp32 tile
    (all 8 banks) treated as a ring of 4 unit-slots of [128, 1024]
    (unit = 1024 candidates x one query-half); unit i lives at slot i%4.
  - Evacuation is split across both PSUM-capable engines (cost-model
    balanced):
      * DVE units: tensor_reduce(max) FD=1024 ((1024+120)/0.96 = 1192 ns)
        -> 8 per-128-candidate chunk leaders per unit.
      * ACT units: activation(Exp, scale=beta, accum) over one unit or a
        PAIR of same-half units ((2048+222)/1.2 = 1892 ns, i.e. 946/unit)
        -> one log-sum-exp leader per evac op.
    The schedule (unit stream order, engine assignment, pairing) is chosen
    to keep slot handoffs crossing engines; see make_plan_* families.
  - The host selects top-K3 chunks per (query, core) among DVE chunk
    leaders and top-KA LSE entries among ACT ops, rescores the gathered
    candidates exactly in fp32, and selects the global top-k.
"""

import numpy as np
import ml_dtypes

# ---- hardcoded problem geometry (nn_CandidateRetriever, spec.json) ----
B = 256            # queries
D = 64             # embedding dim
N = 1000000        # table rows
NCORES = 8
GROUPS = 122       # 1024-candidate groups per core (no padding)
SH = GROUPS * 1024  # 124928 rows per core shard
NDEV = NCORES * SH  # 999424 rows scored on device
CH = 128           # leaf chunk size within DVE units
K3 = 32            # DVE chunks selected per (query, core)
KA = 16            # ACT entries selected per (query, core)
BETA = 128.0       # LSE sharpness
CLSE = 0.6         # LSE centering constant
NEG = -1.0e30

COST_A1 = 1259.0   # ns, ACT single effective (processing+tail+decode)
COST_A2 = 2148.0   # ns, ACT pair effective
COST_D1 = 1284.0   # ns, DVE single effective


def plan_f1(step=2):
    """hb-stream [g0h0,g1h0,g0h1,g1h1]; ACT evacs adjacent-slot pairs;
    greedy deficit in steps of `step` half-blocks (AADD-style)."""
    units = []
    hbs = []
    for i in range(GROUPS // 2):
        for h in ((0, 1) if i % 2 == 0 else (1, 0)):
            hbs.append((2 * i, 2 * i + 1, h))
    evacs = []
    ta = td = 0.0
    for j in range(0, len(hbs), step):
        chunk = hbs[j:j + step]
        a = ta + COST_A2 * len(chunk) <= td + COST_D1 * 2 * len(chunk)
        for g0, g1, h in chunk:
            u = len(units)
            units.append((g0, h))
            units.append((g1, h))
            if a:
                evacs.append(("A", [u, u + 1]))
                ta += COST_A2
            else:
                evacs.append(("D", [u]))
                evacs.append(("D", [u + 1]))
                td += 2 * COST_D1
    return units, evacs


def plan_f2(nY=7):
    """by-group stream [g0h0,g0h1,g1h0,g1h1]; rings of 4 units.
    X-ring: ACT pair on one half at slots (0,2) or (1,3), DVE singles on
    the other half; Y-ring (every ~len/nY): all-ACT (two pairs)."""
    units = []
    evacs = []
    nrings = GROUPS // 2
    ys = set(np.linspace(0, nrings - 1, nY).round().astype(int)) if nY \
        else set()
    for i in range(nrings):
        g0, g1 = 2 * i, 2 * i + 1
        u = len(units)
        units += [(g0, 0), (g0, 1), (g1, 0), (g1, 1)]
        if i in ys:
            evacs.append(("A", [u, u + 2]))
            evacs.append(("A", [u + 1, u + 3]))
        elif i % 2 == 0:
            evacs.append(("D", [u + 1]))
            evacs.append(("A", [u, u + 2]))
            evacs.append(("D", [u + 3]))
        else:
            evacs.append(("D", [u]))
            evacs.append(("A", [u + 1, u + 3]))
            evacs.append(("D", [u + 2]))
    return units, evacs


def plan_f3(step=2, cost_a=COST_A1, cost_d=COST_D1):
    """singles on both engines, greedy deficit per `step` units."""
    units = []
    for i in range(GROUPS // 2):
        for h in ((0, 1) if i % 2 == 0 else (1, 0)):
            units.append((2 * i, h))
            units.append((2 * i + 1, h))
    evacs = []
    ta = td = 0.0
    for j in range(0, len(units), step):
        k = min(step, len(units) - j)
        if ta + cost_a * k <= td + cost_d * k:
            for u in range(j, j + k):
                evacs.append(("A", [u]))
            ta += cost_a * k
        else:
            for u in range(j, j + k):
                evacs.append(("D", [u]))
            td += cost_d * k
    return units, evacs


PLAN_FAMILY = ("f3", {"step": 4})


def make_plan():
    fam, kw = PLAN_FAMILY
    return {"f1": plan_f1, "f2": plan_f2, "f3": plan_f3}[fam](**kw)


PLAN_UNITS, PLAN_EVACS = make_plan()
NACT = sum(1 for e, _ in PLAN_EVACS if e == "A")
NDVE = sum(1 for e, _ in PLAN_EVACS if e == "D")

_compiled_nc = None


def _build_kernel(rep=None, plan=None, inplace_junk=True, xbufs=8,
                  spill_eng="gpsimd"):
    """Build the device kernel. rep=None: single-shot (production).
    rep=N: wrap the body in a hardware For_i loop (timing harness).
    inplace_junk: ACT writes its exp output back over the PSUM region it
    reads (scores are dead after evac) instead of an SBUF junk buffer --
    cuts the modeled SBUF-access init (444 -> 344 cycles) per ACT op."""
    import contextlib

    import concourse.bacc as bacc
    import concourse.mybir as mybir
    from concourse.tile import TileContext

    nc = bacc.Bacc(None, target_bir_lowering=False)

    units, evacs = plan if plan is not None else (PLAN_UNITS, PLAN_EVACS)
    nact = sum(1 for e, _ in evacs if e == "A")
    ndve = sum(1 for e, _ in evacs if e == "D")
    # evac emission point: after its last unit's matmuls
    emit_at = {}
    for i, (eng, us) in enumerate(evacs):
        emit_at.setdefault(max(us), []).append(i)

    xp = nc.declare_dram_parameter("xp", [GROUPS, 128, 512], mybir.dt.bfloat16,
                                   isOutput=False)
    # qT for both query halves, duplicated across both partition halves:
    # qt[p, h*128+m] = qn[h*128+m, p % 64]
    qt = nc.declare_dram_parameter("qt", [128, 256], mybir.dt.bfloat16,
                                   isOutput=False)
    # DVE chunk leaders: col 8*d + j = leader of chunk j (of 8) of the d-th
    # DVE evac op (plan order), for query h*128+p
    mo = nc.declare_dram_parameter("mo", [128, max(ndve * 8, 1)],
                                   mybir.dt.float32, isOutput=True)
    # ACT LSE accumulators: col a = sum(exp(BETA*s - BETA*CLSE)) over the
    # a-th ACT evac op's candidates, for query h*128+p
    ao = nc.declare_dram_parameter("ao", [128, max(nact, 1)],
                                   mybir.dt.float32, isOutput=True)

    with TileContext(nc) as tc:
        with (
            tc.tile_pool(name="const", bufs=1) as cpool,
            tc.tile_pool(name="x", bufs=xbufs) as xpool,
            tc.tile_pool(name="ps1", bufs=1, space="PSUM") as pspool,
        ):
            # queries (both halves, transposed, bf16, partition-duplicated)
            qtile = cpool.tile([128, 256], mybir.dt.bfloat16)
            nc.sync.dma_start(out=qtile[:], in_=qt[:])
            M = cpool.tile([128, max(ndve * 8, 1)], mybir.dt.float32)
            A = cpool.tile([128, max(nact, 1)], mybir.dt.float32)
            bias = cpool.tile([128, 1], mybir.dt.float32)
            nc.vector.memset(bias[:], -BETA * CLSE)
            junk = cpool.tile([128, 2048], mybir.dt.bfloat16)
            ps = pspool.tile([128, 4096], mybir.dt.float32)

            loop_cm = tc.For_i(0, rep, 1) if rep is not None \
                else contextlib.nullcontext()
            with loop_cm:
                nun = len(units)
                SPILL_AFTER = {nun // 2, (nun * 3) // 4, (nun * 7) // 8,
                               nun - 5, nun - 1}
                m_spill = a_spill = 0
                a_i = d_i = 0
                xt_cache = {}

                def get_xt(g):
                    if g not in xt_cache:
                        xt = xpool.tile([128, 512], mybir.dt.bfloat16,
                                        name="xt", tag="xt")
                        nc.sync.dma_start(out=xt[:], in_=xp[g])
                        xt_cache[g] = xt
                    return xt_cache[g]

                last_use = {}
                for u, (g, h) in enumerate(units):
                    last_use[g] = u

                def psum_ap(us):
                    offs = sorted((u % 4) * 1024 for u in us)
                    if len(offs) == 1:
                        return ps[:, offs[0]:offs[0] + 1024]
                    gap = (offs[1] - offs[0]) // 1024
                    r = ps[:, offs[0]:offs[1] + 1024].rearrange(
                        "p (t x) -> p t x", x=1024)
                    return r[:, ::gap, :] if gap > 1 else r

                for u, (g, h) in enumerate(units):
                    xt = get_xt(g)
                    off = (u % 4) * 1024
                    # scores: out[q, cand]; lhsT = qT half [64, 128];
                    # rhs = table^T sub-tile [64, 512]
                    nc.tensor.matmul(
                        ps[:, off:off + 512],
                        qtile[0:64, h * 128:(h + 1) * 128],
                        xt[0:64, :], start=True, stop=True,
                        tile_position=(0, 0))
                    nc.tensor.matmul(
                        ps[:, off + 512:off + 1024],
                        qtile[64:128, h * 128:(h + 1) * 128],
                        xt[64:128, :], start=True, stop=True,
                        tile_position=(64, 0))
                    if last_use[g] == u:
                        xt_cache.pop(g, None)
                    for ei in emit_at.get(u, []):
                        eng, us = evacs[ei]
                        ap = psum_ap(us)
                        if eng == "A":
                            nc.scalar.activation(
                                out=ap if inplace_junk
                                else junk[:, 0:1024 * len(us)],
                                in_=ap,
                                func=mybir.ActivationFunctionType.Exp,
                                scale=BETA, bias=bias[:],
                                accum_out=A[:, a_i:a_i + 1])
                            a_i += 1
                        else:
                            nc.vector.tensor_reduce(
                                M[:, d_i * 8:(d_i + 1) * 8],
                                ap.rearrange("p (c e) -> p c e", e=CH)
                                if len(us) == 1 else ap,
                                axis=mybir.AxisListType.X,
                                op=mybir.AluOpType.max)
                            d_i += 1
                    if u in SPILL_AFTER:
                        dma_eng = getattr(nc, spill_eng, nc.sync)
                        if d_i * 8 > m_spill:
                            dma_eng.dma_start(out=mo[:, m_spill:d_i * 8],
                                              in_=M[:, m_spill:d_i * 8])
                            m_spill = d_i * 8
                        if a_i > a_spill:
                            dma_eng.dma_start(out=ao[:, a_spill:a_i],
                                              in_=A[:, a_spill:a_i])
                            a_spill = a_i

    nc.compile()
    return nc


def _get_nc():
    global _compiled_nc
    if _compiled_nc is None:
        _compiled_nc = _build_kernel()
    return _compiled_nc


def prepare_inputs(q, T):
    """Normalize, cast to bf16, shard and pack per-core device inputs."""
    qn = q / np.maximum(np.sqrt((q * q).sum(-1, keepdims=True)), 1e-12)
    Tn = T / np.maximum(np.sqrt((T * T).sum(-1, keepdims=True)), 1e-12)

    qb = qn.astype(ml_dtypes.bfloat16)
    qtT_h = qb.reshape(2, 128, D).transpose(0, 2, 1)   # [2, 64, 128]
    qtT = np.ascontiguousarray(
        np.tile(np.concatenate([qtT_h[0], qtT_h[1]], axis=1),
                (2, 1)))                               # [128, 256]

    Tb = Tn.astype(ml_dtypes.bfloat16)

    in_maps = []
    for d in range(NCORES):
        Td = Tb[d * SH:(d + 1) * SH]                   # [SH, 64]
        R = Td.reshape(GROUPS, 2, 512, D)              # [g, ab, j, d]
        Xp = np.ascontiguousarray(
            R.transpose(0, 1, 3, 2).reshape(GROUPS, 128, 512))
        in_maps.append({"xp": Xp, "qt": qtT})
    return qn, Tn, in_maps


def kernel(query_embedding, movie_tag_embeddings, k):
    from concourse.bass_utils import run_bass_kernel_spmd

    q = np.ascontiguousarray(np.asarray(query_embedding, dtype=np.float32))
    T = np.ascontiguousarray(np.asarray(movie_tag_embeddings,
                                        dtype=np.float32))
    k = int(k)
    assert q.shape == (B, D) and T.shape == (N, D) and 1 <= k <= 100

    qn, Tn, in_maps = prepare_inputs(q, T)

    nc = _get_nc()
    res = run_bass_kernel_spmd(nc, in_maps, list(range(NCORES)))

    # column maps from the plan
    units, evacs = PLAN_UNITS, PLAN_EVACS
    dve_h = {0: [], 1: []}      # (mo col base, group)
    act_h = {0: [], 1: []}      # (ao col, [groups])
    d_i = a_i = 0
    for eng, us in evacs:
        h = units[us[0]][1]
        if eng == "A":
            act_h[h].append((a_i, [units[u][0] for u in us]))
            a_i += 1
        else:
            dve_h[h].append((8 * d_i, units[us[0]][0]))
            d_i += 1

    # ---- host phase A: DVE chunk leaders -> top-K3 chunks/(q, core) ----
    nA = {h: len(dve_h[h]) * 8 for h in (0, 1)}
    a_rows = np.empty((B, NCORES * K3 * CH), dtype=np.int64)
    chunk_base_h = {}
    col_idx_h = {}
    for h in (0, 1):
        cb = np.empty(nA[h], dtype=np.int64)
        ci = np.empty(nA[h], dtype=np.int64)
        for i, (cb8, g) in enumerate(dve_h[h]):
            cb[i * 8:(i + 1) * 8] = g * 1024 + np.arange(8) * CH
            ci[i * 8:(i + 1) * 8] = cb8 + np.arange(8)
        chunk_base_h[h] = cb
        col_idx_h[h] = ci

    for d in range(NCORES):
        MO = res.results[d]["mo"].astype(np.float32)   # [128, ndve*8]
        for h in (0, 1):
            L = MO[:, col_idx_h[h]]                    # [128, nA]
            ids = np.argpartition(-L, K3, axis=1)[:, :K3]
            rows = (d * SH + chunk_base_h[h][ids][:, :, None] +
                    np.arange(CH)[None, None, :])
            a_rows[h * 128:(h + 1) * 128,
                   d * K3 * CH:(d + 1) * K3 * CH] = rows.reshape(128, -1)

    # ---- phase A rescore (exact fp32), batched over queries ----
    a_scores = np.empty((B, NCORES * K3 * CH), dtype=np.float32)
    QB = 32
    for q0 in range(0, B, QB):
        rows = a_rows[q0:q0 + QB]
        vecs = Tn[rows]
        a_scores[q0:q0 + QB] = np.einsum(
            "qmd,qd->qm", vecs, qn[q0:q0 + QB], dtype=np.float32)

    # ---- phase B: per (core, ACT entry) BLAS GEMM over selecting queries --
    RW = 2048  # max rows per ACT entry
    ka = {h: min(KA, len(act_h[h])) for h in (0, 1)}
    b_scores_full = np.full((B, NCORES * max(ka.values()) * RW), NEG,
                            dtype=np.float32)
    b_rows_full = np.zeros_like(b_scores_full, dtype=np.int64)
    for d in range(NCORES):
        AO = res.results[d]["ao"].astype(np.float64)   # [128, nact]
        for h in (0, 1):
            if not act_h[h]:
                continue
            nP = len(act_h[h])
            cols = [a for a, _ in act_h[h]]
            G = np.log(np.maximum(AO[:, cols], 1e-300)) / BETA + CLSE
            sel = np.argpartition(-G, ka[h] - 1, axis=1)[:, :ka[h]]
            ent_rows = []
            for _, gs in act_h[h]:
                r = np.concatenate(
                    [d * SH + g * 1024 + np.arange(1024) for g in gs])
                ent_rows.append(r)
            qsel = [[] for _ in range(nP)]
            for p in range(128):
                for s in sel[p]:
                    qsel[s].append(p)
            slot_ctr = np.zeros(128, dtype=np.int64)
            base = d * max(ka.values()) * RW
            for i in range(nP):
                qs = qsel[i]
                if not qs:
                    continue
                qs = np.asarray(qs, dtype=np.int64)
                rows = ent_rows[i]
                Sg = qn[h * 128 + qs] @ Tn[rows].T     # [nq, nr] fp32
                slots = slot_ctr[qs]
                for j, (p, sl) in enumerate(zip(qs, slots)):
                    c0 = base + sl * RW
                    b_scores_full[h * 128 + p, c0:c0 + rows.size] = Sg[j]
                    b_rows_full[h * 128 + p, c0:c0 + rows.size] = rows
                slot_ctr[qs] += 1

    # ---- remainder rows (not on device): exact host scoring ----
    rem_rows = np.arange(NDEV, N, dtype=np.int64)
    rem_scores = (qn @ Tn[rem_rows].T).astype(np.float32)  # [B, 576]
    rem_rows_b = np.broadcast_to(rem_rows, (B, rem_rows.size))

    # ---- merge + global top-k (reference tie-break: desc value, asc idx) --
    all_scores = np.concatenate(
        [a_scores, b_scores_full, rem_scores], axis=1)
    all_rows = np.concatenate(
        [a_rows, b_rows_full, rem_rows_b], axis=1)
    m = k + 8
    part = np.argpartition(-all_scores, m, axis=1)[:, :m]
    pv = np.take_along_axis(all_scores, part, axis=1)
    pr = np.take_along_axis(all_rows, part, axis=1)
    order = np.lexsort((pr, -pv), axis=1)[:, :k]
    top_vals = np.take_along_axis(pv, order, axis=1).astype(np.float32)
    top_idx = np.take_along_axis(pr, order, axis=1).astype(np.int32)
    return top_vals, top_idx


# revision 15
# speedup vs baseline: 1.4558x; 1.4558x over previous
"""Distributed cosine-similarity top-k retrieval kernel for 8 Trainium2 NeuronCores.

Strategy (sharding_hint: row-wise table sharding):
  - 999424 of the 1M table rows are L2-normalized and sharded row-wise across
    8 cores (124928 rows = 122 groups of 1024 candidates each); the 576
    remainder rows are scored exactly on the host and merged.
  - Each core streams its shard through the TensorEngine (bf16 matmul vs all
    256 queries -> fp32 scores in PSUM). PSUM is one [128, 4096] fp32 tile
    (all 8 banks) treated as a ring of 4 unit-slots of [128, 1024]
    (unit = 1024 candidates x one query-half); unit i lives at slot i%4.
  - Evacuation is split across both PSUM-capable engines (cost-model
    balanced):
      * DVE units: tensor_reduce(max) FD=1024 ((1024+120)/0.96 = 1192 ns)
        -> 8 per-128-candidate chunk leaders per unit.
      * ACT units: activation(Exp, scale=beta, accum) over one unit or a
        PAIR of same-half units ((2048+222)/1.2 = 1892 ns, i.e. 946/unit)
        -> one log-sum-exp leader per evac op.
    The schedule (unit stream order, engine assignment, pairing) is chosen
    to keep slot handoffs crossing engines; see make_plan_* families.
  - The host selects top-K3 chunks per (query, core) among DVE chunk
    leaders and top-KA LSE entries among ACT ops, rescores the gathered
    candidates exactly in fp32, and selects the global top-k.
"""

import numpy as np
import ml_dtypes

# ---- hardcoded problem geometry (nn_CandidateRetriever, spec.json) ----
B = 256            # queries
D = 64             # embedding dim
N = 1000000        # table rows
NCORES = 8
GROUPS = 122       # 1024-candidate groups per core (no padding)
SH = GROUPS * 1024  # 124928 rows per core shard
NDEV = NCORES * SH  # 999424 rows scored on device
CH = 128           # leaf chunk size within DVE units
K3 = 32            # DVE chunks selected per (query, core)
KA = 16            # ACT entries selected per (query, core)
BETA = 128.0       # LSE sharpness
CLSE = 0.6         # LSE centering constant
NEG = -1.0e30

COST_A1 = 1223.6   # ns, ACT single in-place, HW-measured back-to-back
COST_A2 = 2041.1   # ns, ACT pair in-place, HW-measured
COST_D1 = 1172.7   # ns, DVE single, HW-measured


def plan_f1(step=2):
    """hb-stream [g0h0,g1h0,g0h1,g1h1]; ACT evacs adjacent-slot pairs;
    greedy deficit in steps of `step` half-blocks (AADD-style)."""
    units = []
    hbs = []
    for i in range(GROUPS // 2):
        for h in ((0, 1) if i % 2 == 0 else (1, 0)):
            hbs.append((2 * i, 2 * i + 1, h))
    evacs = []
    ta = td = 0.0
    for j in range(0, len(hbs), step):
        chunk = hbs[j:j + step]
        a = ta + COST_A2 * len(chunk) <= td + COST_D1 * 2 * len(chunk)
        for g0, g1, h in chunk:
            u = len(units)
            units.append((g0, h))
            units.append((g1, h))
            if a:
                evacs.append(("A", [u, u + 1]))
                ta += COST_A2
            else:
                evacs.append(("D", [u]))
                evacs.append(("D", [u + 1]))
                td += 2 * COST_D1
    return units, evacs


def plan_f2(nY=7):
    """by-group stream [g0h0,g0h1,g1h0,g1h1]; rings of 4 units.
    X-ring: ACT pair on one half at slots (0,2) or (1,3), DVE singles on
    the other half; Y-ring (every ~len/nY): all-ACT (two pairs)."""
    units = []
    evacs = []
    nrings = GROUPS // 2
    ys = set(np.linspace(0, nrings - 1, nY).round().astype(int)) if nY \
        else set()
    for i in range(nrings):
        g0, g1 = 2 * i, 2 * i + 1
        u = len(units)
        units += [(g0, 0), (g0, 1), (g1, 0), (g1, 1)]
        if i in ys:
            evacs.append(("A", [u, u + 2]))
            evacs.append(("A", [u + 1, u + 3]))
        elif i % 2 == 0:
            evacs.append(("D", [u + 1]))
            evacs.append(("A", [u, u + 2]))
            evacs.append(("D", [u + 3]))
        else:
            evacs.append(("D", [u]))
            evacs.append(("A", [u + 1, u + 3]))
            evacs.append(("D", [u + 2]))
    return units, evacs


def plan_f3(step=2, cost_a=COST_A1, cost_d=COST_D1):
    """singles on both engines, greedy deficit per `step` units."""
    units = []
    for i in range(GROUPS // 2):
        for h in ((0, 1) if i % 2 == 0 else (1, 0)):
            units.append((2 * i, h))
            units.append((2 * i + 1, h))
    evacs = []
    ta = td = 0.0
    for j in range(0, len(units), step):
        k = min(step, len(units) - j)
        if ta + cost_a * k <= td + cost_d * k:
            for u in range(j, j + k):
                evacs.append(("A", [u]))
            ta += cost_a * k
        else:
            for u in range(j, j + k):
                evacs.append(("D", [u]))
            td += cost_d * k
    return units, evacs


PLAN_FAMILY = ("f3", {"step": 4})


def make_plan():
    fam, kw = PLAN_FAMILY
    return {"f1": plan_f1, "f2": plan_f2, "f3": plan_f3}[fam](**kw)


PLAN_UNITS, PLAN_EVACS = make_plan()
NACT = sum(1 for e, _ in PLAN_EVACS if e == "A")
NDVE = sum(1 for e, _ in PLAN_EVACS if e == "D")

_compiled_nc = None


def _build_kernel(rep=None, plan=None, inplace_junk=True, xbufs=8,
                  spill_eng="gpsimd"):
    """Build the device kernel. rep=None: single-shot (production).
    rep=N: wrap the body in a hardware For_i loop (timing harness).
    inplace_junk: ACT writes its exp output back over the PSUM region it
    reads (scores are dead after evac) instead of an SBUF junk buffer --
    cuts the modeled SBUF-access init (444 -> 344 cycles) per ACT op."""
    import contextlib

    import concourse.bacc as bacc
    import concourse.mybir as mybir
    from concourse.tile import TileContext

    nc = bacc.Bacc(None, target_bir_lowering=False)

    units, evacs = plan if plan is not None else (PLAN_UNITS, PLAN_EVACS)
    nact = sum(1 for e, _ in evacs if e == "A")
    ndve = sum(1 for e, _ in evacs if e == "D")
    # evac emission point: after its last unit's matmuls
    emit_at = {}
    for i, (eng, us) in enumerate(evacs):
        emit_at.setdefault(max(us), []).append(i)

    xp = nc.declare_dram_parameter("xp", [GROUPS, 128, 512], mybir.dt.bfloat16,
                                   isOutput=False)
    # qT for both query halves, duplicated across both partition halves:
    # qt[p, h*128+m] = qn[h*128+m, p % 64]
    qt = nc.declare_dram_parameter("qt", [128, 256], mybir.dt.bfloat16,
                                   isOutput=False)
    # DVE chunk leaders: col 8*d + j = leader of chunk j (of 8) of the d-th
    # DVE evac op (plan order), for query h*128+p
    mo = nc.declare_dram_parameter("mo", [128, max(ndve * 8, 1)],
                                   mybir.dt.float32, isOutput=True)
    # ACT LSE accumulators: col a = sum(exp(BETA*s - BETA*CLSE)) over the
    # a-th ACT evac op's candidates, for query h*128+p
    ao = nc.declare_dram_parameter("ao", [128, max(nact, 1)],
                                   mybir.dt.float32, isOutput=True)

    with TileContext(nc) as tc:
        with (
            tc.tile_pool(name="const", bufs=1) as cpool,
            tc.tile_pool(name="x", bufs=xbufs) as xpool,
            tc.tile_pool(name="ps1", bufs=1, space="PSUM") as pspool,
        ):
            # queries (both halves, transposed, bf16, partition-duplicated)
            qtile = cpool.tile([128, 256], mybir.dt.bfloat16)
            nc.sync.dma_start(out=qtile[:], in_=qt[:])
            M = cpool.tile([128, max(ndve * 8, 1)], mybir.dt.float32)
            A = cpool.tile([128, max(nact, 1)], mybir.dt.float32)
            bias = cpool.tile([128, 1], mybir.dt.float32)
            nc.vector.memset(bias[:], -BETA * CLSE)
            junk = cpool.tile([128, 2048], mybir.dt.bfloat16)
            ps = pspool.tile([128, 4096], mybir.dt.float32)
            # force the exp table load at kernel start (overlaps DMA waits)
            nc.scalar.activation(
                out=junk[:, 0:1], in_=bias[:],
                func=mybir.ActivationFunctionType.Exp, scale=0.0)

            loop_cm = tc.For_i(0, rep, 1) if rep is not None \
                else contextlib.nullcontext()
            with loop_cm:
                nun = len(units)
                SPILL_AFTER = {nun // 2, (nun * 3) // 4, (nun * 7) // 8,
                               nun - 5, nun - 1}
                m_spill = a_spill = 0
                a_i = d_i = 0
                xt_cache = {}

                def get_xt(g):
                    if g not in xt_cache:
                        xt = xpool.tile([128, 512], mybir.dt.bfloat16,
                                        name="xt", tag="xt")
                        nc.sync.dma_start(out=xt[:], in_=xp[g])
                        xt_cache[g] = xt
                    return xt_cache[g]

                last_use = {}
                for u, (g, h) in enumerate(units):
                    last_use[g] = u

                def psum_ap(us):
                    offs = sorted((u % 4) * 1024 for u in us)
                    if len(offs) == 1:
                        return ps[:, offs[0]:offs[0] + 1024]
                    gap = (offs[1] - offs[0]) // 1024
                    r = ps[:, offs[0]:offs[1] + 1024].rearrange(
                        "p (t x) -> p t x", x=1024)
                    return r[:, ::gap, :] if gap > 1 else r

                for u, (g, h) in enumerate(units):
                    xt = get_xt(g)
                    off = (u % 4) * 1024
                    # scores: out[q, cand]; lhsT = qT half [64, 128];
                    # rhs = table^T sub-tile [64, 512]
                    nc.tensor.matmul(
                        ps[:, off:off + 512],
                        qtile[0:64, h * 128:(h + 1) * 128],
                        xt[0:64, :], start=True, stop=True,
                        tile_position=(0, 0))
                    nc.tensor.matmul(
                        ps[:, off + 512:off + 1024],
                        qtile[64:128, h * 128:(h + 1) * 128],
                        xt[64:128, :], start=True, stop=True,
                        tile_position=(64, 0))
                    if last_use[g] == u:
                        xt_cache.pop(g, None)
                    for ei in emit_at.get(u, []):
                        eng, us = evacs[ei]
                        ap = psum_ap(us)
                        if eng == "A":
                            nc.scalar.activation(
                                out=ap if inplace_junk
                                else junk[:, 0:1024 * len(us)],
                                in_=ap,
                                func=mybir.ActivationFunctionType.Exp,
                                scale=BETA, bias=bias[:],
                                accum_out=A[:, a_i:a_i + 1])
                            a_i += 1
                        else:
                            nc.vector.tensor_reduce(
                                M[:, d_i * 8:(d_i + 1) * 8],
                                ap.rearrange("p (c e) -> p c e", e=CH)
                                if len(us) == 1 else ap,
                                axis=mybir.AxisListType.X,
                                op=mybir.AluOpType.max)
                            d_i += 1
                    if u in SPILL_AFTER:
                        dma_eng = getattr(nc, spill_eng, nc.sync)
                        if d_i * 8 > m_spill:
                            dma_eng.dma_start(out=mo[:, m_spill:d_i * 8],
                                              in_=M[:, m_spill:d_i * 8])
                            m_spill = d_i * 8
                        if a_i > a_spill:
                            dma_eng.dma_start(out=ao[:, a_spill:a_i],
                                              in_=A[:, a_spill:a_i])
                            a_spill = a_i

    nc.compile()
    return nc


def _get_nc():
    global _compiled_nc
    if _compiled_nc is None:
        _compiled_nc = _build_kernel()
    return _compiled_nc


def prepare_inputs(q, T):
    """Normalize, cast to bf16, shard and pack per-core device inputs."""
    qn = q / np.maximum(np.sqrt((q * q).sum(-1, keepdims=True)), 1e-12)
    Tn = T / np.maximum(np.sqrt((T * T).sum(-1, keepdims=True)), 1e-12)

    qb = qn.astype(ml_dtypes.bfloat16)
    qtT_h = qb.reshape(2, 128, D).transpose(0, 2, 1)   # [2, 64, 128]
    qtT = np.ascontiguousarray(
        np.tile(np.concatenate([qtT_h[0], qtT_h[1]], axis=1),
                (2, 1)))                               # [128, 256]

    Tb = Tn.astype(ml_dtypes.bfloat16)

    in_maps = []
    for d in range(NCORES):
        Td = Tb[d * SH:(d + 1) * SH]                   # [SH, 64]
        R = Td.reshape(GROUPS, 2, 512, D)              # [g, ab, j, d]
        Xp = np.ascontiguousarray(
            R.transpose(0, 1, 3, 2).reshape(GROUPS, 128, 512))
        in_maps.append({"xp": Xp, "qt": qtT})
    return qn, Tn, in_maps


def kernel(query_embedding, movie_tag_embeddings, k):
    from concourse.bass_utils import run_bass_kernel_spmd

    q = np.ascontiguousarray(np.asarray(query_embedding, dtype=np.float32))
    T = np.ascontiguousarray(np.asarray(movie_tag_embeddings,
                                        dtype=np.float32))
    k = int(k)
    assert q.shape == (B, D) and T.shape == (N, D) and 1 <= k <= 100

    qn, Tn, in_maps = prepare_inputs(q, T)

    nc = _get_nc()
    res = run_bass_kernel_spmd(nc, in_maps, list(range(NCORES)))

    # column maps from the plan
    units, evacs = PLAN_UNITS, PLAN_EVACS
    dve_h = {0: [], 1: []}      # (mo col base, group)
    act_h = {0: [], 1: []}      # (ao col, [groups])
    d_i = a_i = 0
    for eng, us in evacs:
        h = units[us[0]][1]
        if eng == "A":
            act_h[h].append((a_i, [units[u][0] for u in us]))
            a_i += 1
        else:
            dve_h[h].append((8 * d_i, units[us[0]][0]))
            d_i += 1

    # ---- host phase A: DVE chunk leaders -> top-K3 chunks/(q, core) ----
    nA = {h: len(dve_h[h]) * 8 for h in (0, 1)}
    a_rows = np.empty((B, NCORES * K3 * CH), dtype=np.int64)
    chunk_base_h = {}
    col_idx_h = {}
    for h in (0, 1):
        cb = np.empty(nA[h], dtype=np.int64)
        ci = np.empty(nA[h], dtype=np.int64)
        for i, (cb8, g) in enumerate(dve_h[h]):
            cb[i * 8:(i + 1) * 8] = g * 1024 + np.arange(8) * CH
            ci[i * 8:(i + 1) * 8] = cb8 + np.arange(8)
        chunk_base_h[h] = cb
        col_idx_h[h] = ci

    for d in range(NCORES):
        MO = res.results[d]["mo"].astype(np.float32)   # [128, ndve*8]
        for h in (0, 1):
            L = MO[:, col_idx_h[h]]                    # [128, nA]
            ids = np.argpartition(-L, K3, axis=1)[:, :K3]
            rows = (d * SH + chunk_base_h[h][ids][:, :, None] +
                    np.arange(CH)[None, None, :])
            a_rows[h * 128:(h + 1) * 128,
                   d * K3 * CH:(d + 1) * K3 * CH] = rows.reshape(128, -1)

    # ---- phase A rescore (exact fp32), batched over queries ----
    a_scores = np.empty((B, NCORES * K3 * CH), dtype=np.float32)
    QB = 32
    for q0 in range(0, B, QB):
        rows = a_rows[q0:q0 + QB]
        vecs = Tn[rows]
        a_scores[q0:q0 + QB] = np.einsum(
            "qmd,qd->qm", vecs, qn[q0:q0 + QB], dtype=np.float32)

    # ---- phase B: per (core, ACT entry) BLAS GEMM over selecting queries --
    RW = 2048  # max rows per ACT entry
    ka = {h: min(KA, len(act_h[h])) for h in (0, 1)}
    b_scores_full = np.full((B, NCORES * max(ka.values()) * RW), NEG,
                            dtype=np.float32)
    b_rows_full = np.zeros_like(b_scores_full, dtype=np.int64)
    for d in range(NCORES):
        AO = res.results[d]["ao"].astype(np.float64)   # [128, nact]
        for h in (0, 1):
            if not act_h[h]:
                continue
            nP = len(act_h[h])
            cols = [a for a, _ in act_h[h]]
            G = np.log(np.maximum(AO[:, cols], 1e-300)) / BETA + CLSE
            sel = np.argpartition(-G, ka[h] - 1, axis=1)[:, :ka[h]]
            ent_rows = []
            for _, gs in act_h[h]:
                r = np.concatenate(
                    [d * SH + g * 1024 + np.arange(1024) for g in gs])
                ent_rows.append(r)
            qsel = [[] for _ in range(nP)]
            for p in range(128):
                for s in sel[p]:
                    qsel[s].append(p)
            slot_ctr = np.zeros(128, dtype=np.int64)
            base = d * max(ka.values()) * RW
            for i in range(nP):
                qs = qsel[i]
                if not qs:
                    continue
                qs = np.asarray(qs, dtype=np.int64)
                rows = ent_rows[i]
                Sg = qn[h * 128 + qs] @ Tn[rows].T     # [nq, nr] fp32
                slots = slot_ctr[qs]
                for j, (p, sl) in enumerate(zip(qs, slots)):
                    c0 = base + sl * RW
                    b_scores_full[h * 128 + p, c0:c0 + rows.size] = Sg[j]
                    b_rows_full[h * 128 + p, c0:c0 + rows.size] = rows
                slot_ctr[qs] += 1

    # ---- remainder rows (not on device): exact host scoring ----
    rem_rows = np.arange(NDEV, N, dtype=np.int64)
    rem_scores = (qn @ Tn[rem_rows].T).astype(np.float32)  # [B, 576]
    rem_rows_b = np.broadcast_to(rem_rows, (B, rem_rows.size))

    # ---- merge + global top-k (reference tie-break: desc value, asc idx) --
    all_scores = np.concatenate(
        [a_scores, b_scores_full, rem_scores], axis=1)
    all_rows = np.concatenate(
        [a_rows, b_rows_full, rem_rows_b], axis=1)
    m = k + 8
    part = np.argpartition(-all_scores, m, axis=1)[:, :m]
    pv = np.take_along_axis(all_scores, part, axis=1)
    pr = np.take_along_axis(all_rows, part, axis=1)
    order = np.lexsort((pr, -pv), axis=1)[:, :k]
    top_vals = np.take_along_axis(pv, order, axis=1).astype(np.float32)
    top_idx = np.take_along_axis(pr, order, axis=1).astype(np.int32)
    return top_vals, top_idx
